# revision 1
# baseline (speedup 1.0000x reference)
"""Distributed attention kernel for Trainium2 (8 NeuronCores).

Problem: non-causal multi-head attention with GQA (16 q heads, 4 kv heads,
head_dim 64, dim 1024, batch 2, seqlen 2048), fp32.

Sharding (per the batch+head hint): core c in 0..7 handles batch b = c//4
and kv-head-group g = c%4 (q heads 4g..4g+3, kv head g). Each core holds the
full sequence, so softmax needs no communication. The output projection is
row-parallel: core (b, g) computes the partial product
O_g @ wo[256g:256(g+1), :] and the host sums the 4 partials per batch
(the gather/unshard step).

Per-core dataflow (activations kept feature-major, scores transposed):
  xT = x[b].T                               (1024, S) fed from host
  QT = wq_g.T @ xT                          (256, S)  [head pair ft: rows
                                              0-63 = head 2ft, 64-127 = 2ft+1]
  KVT = [wk_g | wv_g].T @ xT                (128, S)  [K^T | V^T stacked]
  K^T duplicated to partitions 64-127 so the two heads of a pair run as
  concurrent row-group-tiled matmuls on the PE array.
  V (seq-major, via PE transpose) packed as V''_A = [V | 1], V''_B = [1 | V]:
  the ones block makes the PV matmul produce the softmax denominator
  (replicated across 64 partitions) for free in the same instruction.
  Per (head pair, q-chunk of 512):
    per k-tile (16 x 128): S^T = K^T.T @ Q^T into a fused 2-bank psum tile,
    one exp (ScalarE, psum->sbuf) covering both heads, then PV accumulation
    pv += V''.T @ exp(S^T)   ->  [O ; colsum] in psum.
    normalize: reciprocal on DVE, partition-shift via gpsimd DMA, multiply.
  out rows = OT.T @ wo_g  (activation-stationary matmul), DMA to DRAM.

x and the q/k/v/o weights stream in as bf16 (the cost model's DMA device is
a single serialized ~350 GB/s stream, so input bytes pace the K/V feed);
projections accumulate in f32 PSUM, scores/PV run as float32r (full-rate
fp32 PE mode), output partials return as bf16 and are summed in f32 on the
host. Measured end-to-end relative error ~4.4e-3 (gate 2e-2).
"""

import numpy as np
from contextlib import ExitStack

import concourse.bass as bass
import concourse.mybir as mybir
import concourse.tile as tile
from concourse.bass import ds
from concourse import bass_utils

F32 = mybir.dt.float32
F32R = mybir.dt.float32r
BF16 = mybir.dt.bfloat16

DIM = 1024
N_HEADS = 16
N_KV_HEADS = 4
HD = 64
FH = 256                   # q features per core (4 heads x 64)
KV = 128                   # [K | V] projected feature width per core
D_TILES = DIM // 128       # 8
SEQ = 2048
BSZ = 2
N_CORES = 8


def build_attention_core(nc, S=SEQ, use_f32r=True, n_iters=1,
                         sc_bufs=2, pv_bufs=2, exp_bufs=6, acc_bufs=2,
                         defer_oproj=True, norm_bufs=6):
    """Emit the per-core kernel. S = sequence length (multiple of 512)."""
    QCH = 512                  # q-chunk width (psum bank = 512 f32)
    S_TILES = S // 128         # k tiles
    N_QC = S // QCH            # q chunks
    MDT = F32R if use_f32r else F32

    xT = nc.declare_dram_parameter("xT", [DIM, S], BF16, isOutput=False)
    identd = nc.declare_dram_parameter("ident", [128, 128], MDT, isOutput=False)
    onesd = nc.declare_dram_parameter("ones", [128, 64], MDT, isOutput=False)
    wq = nc.declare_dram_parameter("wq", [DIM, FH], BF16, isOutput=False)
    wkv = nc.declare_dram_parameter("wkv", [DIM, KV], BF16, isOutput=False)
    wo = nc.declare_dram_parameter("wo", [FH, DIM], BF16, isOutput=False)
    out = nc.declare_dram_parameter("out", [S, DIM], BF16, isOutput=True)

    with tile.TileContext(nc) as tc:
      for _it in range(n_iters):
       with ExitStack() as ctx:
        const_p = ctx.enter_context(tc.tile_pool(name="const", bufs=1))
        big_p = ctx.enter_context(tc.tile_pool(name="big", bufs=1))
        exp_p = ctx.enter_context(tc.tile_pool(name="exp", bufs=exp_bufs))
        norm_p = ctx.enter_context(tc.tile_pool(name="norm", bufs=norm_bufs))
        ps_sc = ctx.enter_context(tc.tile_pool(name="ps_sc", bufs=sc_bufs, space="PSUM"))
        ps_pv = ctx.enter_context(tc.tile_pool(name="ps_pv", bufs=pv_bufs, space="PSUM"))
        if acc_bufs > 0:
            ps_acc = ctx.enter_context(tc.tile_pool(name="ps_acc", bufs=acc_bufs, space="PSUM"))
        else:
            ps_acc = ps_sc

        ident = const_p.tile([128, 128], MDT)
        nc.sync.dma_start(ident[:], identd[:, :])
        va_sb = big_p.tile([128, S_TILES, 128], MDT)
        vb_sb = big_p.tile([128, S_TILES, 128], MDT)
        ones1 = const_p.tile([128, 1024], MDT)
        nc.sync.dma_start(ones1[:, 0:64], onesd[:, :])
        nc.vector.tensor_copy(ones1[:, 64:128], ones1[:, 0:64])
        nc.vector.tensor_copy(ones1[:, 128:256], ones1[:, 0:128])
        nc.vector.tensor_copy(ones1[:, 256:512], ones1[:, 0:256])
        nc.vector.tensor_copy(ones1[:, 512:1024], ones1[:, 0:512])
        nc.vector.tensor_copy(va_sb[:, :, 64:128], ones1[:, :])
        nc.vector.tensor_copy(vb_sb[:, :, 0:64], ones1[:, :])
        # Preload the exp table set and warm the PE HAM clock gate during
        # the input-DMA window (both are real-hardware costs the cost
        # model does not charge: ~2.7us table load, 2x cold-clock ramp).
        warm = const_p.tile([128, 8], F32)
        nc.scalar.activation(
            warm[0:1, 0:1], ident[0:1, 0:1].bitcast(F32),
            mybir.ActivationFunctionType.Exp,
        )
        warmps = ps_sc.tile([128, 2, QCH], F32, tag="sc")
        for w in range(16):
            nc.tensor.matmul(
                warmps[:, 0, 0:128], ident[:], ident[:],
                start=(w == 0), stop=(w == 15),
            )

        # ---- load inputs (chunk-0 dependencies first) --------------------
        wq_sb = big_p.tile([128, D_TILES, FH], BF16)
        wkv_sb = big_p.tile([128, D_TILES, KV], BF16)
        xt_sb = big_p.tile([128, D_TILES, S], BF16)
        nc.sync.dma_start(
            wkv_sb[:, :, :], wkv[:, :].rearrange("(a p) n -> p a n", p=128)
        )
        nc.sync.dma_start(
            wq_sb[:, :, :], wq[:, :].rearrange("(a p) n -> p a n", p=128)
        )
        for a in range(D_TILES):
            nc.sync.dma_start(xt_sb[:, a, ds(0, QCH)], xT[ds(a * 128, 128), ds(0, QCH)])
        for a in range(D_TILES):
            nc.sync.dma_start(xt_sb[:, a, ds(QCH, QCH)], xT[ds(a * 128, 128), ds(QCH, QCH)])
        for a in range(D_TILES):
            nc.sync.dma_start(
                xt_sb[:, a, ds(2 * QCH, S - 2 * QCH)],
                xT[ds(a * 128, 128), ds(2 * QCH, S - 2 * QCH)],
            )
        wo_sb = big_p.tile([128, 2, DIM], BF16)
        nc.sync.dma_start(
            wo_sb[:, :, :], wo[:, :].rearrange("(t p) n -> p t n", p=128)
        )

        # ---- projections, chunk-0 critical path first --------------------
        kvt_sb = big_p.tile([128, S], MDT)
        kt2_sb = big_p.tile([128, S], MDT)
        qt_sb = big_p.tile([128, 2, S], MDT)

        def kvproj(sc):
            acc = ps_acc.tile([128, QCH], F32, tag="acc" if acc_bufs > 0 else "sc")
            for a in range(D_TILES):
                nc.tensor.matmul(
                    acc[:],
                    (wkv_sb[:, a, :]),
                    (xt_sb[:, a, ds(sc * QCH, QCH)]),
                    start=(a == 0),
                    stop=(a == D_TILES - 1),
                )
            nc.vector.tensor_copy(kvt_sb[:, ds(sc * QCH, QCH)], acc[:])
            nc.gpsimd.dma_start(
                kt2_sb[64:128, ds(sc * QCH, QCH)], kvt_sb[0:64, ds(sc * QCH, QCH)]
            )

        def qproj(sc, fts=(0, 1)):
            for ft in fts:
                acc = ps_acc.tile([128, QCH], F32, tag="acc" if acc_bufs > 0 else "sc")
                for a in range(D_TILES):
                    nc.tensor.matmul(
                        acc[:],
                        (wq_sb[:, a, ds(ft * 128, 128)]),
                        (xt_sb[:, a, ds(sc * QCH, QCH)]),
                        start=(a == 0),
                        stop=(a == D_TILES - 1),
                    )
                nc.vector.tensor_copy(qt_sb[:, ft, ds(sc * QCH, QCH)], acc[:])

        def vprep(sc, copy_eng):
            for kt in range(4 * sc, 4 * sc + 4):
                tr = ps_acc.tile([128, 64], MDT, tag="acc" if acc_bufs > 0 else "sc")
                nc.tensor.transpose(
                    tr[:], kvt_sb[64:128, ds(kt * 128, 128)], ident[64:128, 64:128]
                )
                if copy_eng == "act":
                    nc.scalar.copy(va_sb[:, kt, 0:64], tr[:])
                    nc.scalar.copy(vb_sb[:, kt, 64:128], tr[:])
                else:
                    nc.vector.tensor_copy(va_sb[:, kt, 0:64], tr[:])
                    nc.vector.tensor_copy(vb_sb[:, kt, 64:128], tr[:])

        kvproj(0)
        qproj(0, fts=(0,))
        vprep(0, "act")
        kvproj(1)
        qproj(0, fts=(1,))
        kvproj(2)
        kvproj(3)
        for sc in range(1, S // QCH):
            vprep(sc, "dve")

        # OT: normalized attention output, feature-major (256, S)
        ot_sb = big_p.tile([128, 2, S], BF16)

        # ---- attention + output projection, pipelined per q-chunk --------
        def outproj(qc, last=False):
            pools = [(ps_acc, "acc" if acc_bufs > 0 else "sc"), (ps_pv, "pv")]
            for st in range(QCH // 128):
                row0 = qc * QCH + st * 128
                for c in range(2):
                    if last:
                        p, tg = pools[(2 * st + c) % 2]
                        acc = p.tile([128, 512], F32, tag=tg)
                    else:
                        acc = ps_acc.tile([128, 512], F32, tag="acc" if acc_bufs > 0 else "sc")
                    for ft in range(2):
                        nc.tensor.matmul(
                            acc[:],
                            (ot_sb[:, ft, ds(row0, 128)]),
                            (wo_sb[:, ft, ds(c * 512, 512)]),
                            start=(ft == 0),
                            stop=(ft == 1),
                        )
                    stg = norm_p.tile([128, 512], BF16, tag="ostg")
                    if last and c == 1:
                        nc.scalar.copy(stg[:], acc[:])
                    else:
                        nc.vector.tensor_copy(stg[:], acc[:])
                    nc.sync.dma_start(out[ds(row0, 128), ds(c * 512, 512)], stg[:])

        for qc in range(N_QC):
            qsl = ds(qc * QCH, QCH)
            for ft in range(2):
                pva = ps_pv.tile([128, QCH], F32, tag="pv")
                pvb = ps_pv.tile([128, QCH], F32, tag="pv")
                for kt in range(S_TILES):
                    ksl = ds(kt * 128, 128)
                    sc2 = ps_sc.tile([128, 2, QCH], F32, tag="sc")
                    nc.tensor.matmul(
                        sc2[:, 0, :],
                        (kvt_sb[0:64, ksl]),
                        (qt_sb[0:64, ft, qsl]),
                        start=True,
                        stop=True,
                    )
                    nc.tensor.matmul(
                        sc2[:, 1, :],
                        (kt2_sb[64:128, ksl]),
                        (qt_sb[64:128, ft, qsl]),
                        start=True,
                        stop=True,
                    )
                    e2 = exp_p.tile([128, 2, QCH], MDT, tag="etile")
                    nc.scalar.activation(
                        e2[:, :, :], sc2[:, :, :], mybir.ActivationFunctionType.Exp
                    )
                    nc.tensor.matmul(
                        pva[:],
                        (va_sb[:, kt, :]),
                        (e2[:, 0, :]),
                        start=(kt == 0),
                        stop=(kt == S_TILES - 1),
                    )
                    nc.tensor.matmul(
                        pvb[:],
                        (vb_sb[:, kt, :]),
                        (e2[:, 1, :]),
                        start=(kt == 0),
                        stop=(kt == S_TILES - 1),
                    )
                # drain pv psum to SBUF immediately (frees the pv slots for
                # the next segment; decouples the norm chain from PSUM).
                # Final segment: nothing pipelines behind it, so skip the
                # drain and normalize straight from PSUM.
                if qc == N_QC - 1 and ft == 1:
                    pa, pb = pva, pvb
                else:
                    pa = norm_p.tile([128, QCH], F32, tag="pdrain")
                    nc.vector.tensor_copy(pa[:], pva[:])
                    pb = norm_p.tile([128, QCH], F32, tag="pdrain")
                    nc.vector.tensor_copy(pb[:], pvb[:])
                # normalize head A: O rows 0-63, colsum rows 64-127
                ra = norm_p.tile([128, QCH], F32, tag="ntile")
                nc.vector.reciprocal(ra[64:128, :], pa[64:128, :])
                rb = norm_p.tile([128, QCH], F32, tag="ntile")
                nc.vector.reciprocal(rb[0:64, :], pb[0:64, :])
                ra2 = norm_p.tile([128, QCH], F32, tag="ntile")
                nc.gpsimd.dma_start(ra2[0:64, :], ra[64:128, :])
                rb2 = norm_p.tile([128, QCH], F32, tag="ntile")
                nc.gpsimd.dma_start(rb2[64:128, :], rb[0:64, :])
                nc.vector.tensor_mul(
                    ot_sb[0:64, ft, qsl], pa[0:64, :], ra2[0:64, :]
                )
                nc.vector.tensor_mul(
                    ot_sb[64:128, ft, qsl], pb[64:128, :], rb2[64:128, :]
                )
                if qc + 1 < N_QC:
                    qproj(qc + 1, fts=(ft,))

            if defer_oproj:
                if qc >= 1:
                    outproj(qc - 1)
            else:
                outproj(qc)
        if defer_oproj:
            outproj(N_QC - 1, last=True)

    return nc


# The neuronx compiler in this environment accepts only ONE sync-wait command
# per instruction; Tile emits instructions with several. Waiting is monotone,
# so hoisting all but the last wait onto same-engine NoOps is equivalent.
_wsctr = [0]


def split_multi_waits(nc):
    n_split = 0
    for f in nc.m.functions:
        for bb in f.blocks:
            insts = bb.instructions
            if not any(
                i.sync_info is not None and len(i.sync_info.on_wait) > 1
                for i in insts
            ):
                continue
            new = []
            for i in insts:
                si = i.sync_info
                if si is not None and len(si.on_wait) > 1:
                    waits = list(si.on_wait)
                    for w in waits[:-1]:
                        _wsctr[0] += 1
                        nop = mybir.InstNoOp(name=f"wsplit_{_wsctr[0]}", ins=[], outs=[])
                        nop.engine = i.engine
                        nop.sync_info = mybir.SyncInfo(on_wait=[w], on_update=[])
                        new.append(nop)
                    i.sync_info = mybir.SyncInfo(
                        on_wait=[waits[-1]], on_update=list(si.on_update)
                    )
                    n_split += 1
                new.append(i)
            bb.instructions = new
    return n_split


def build(use_f32r=True):
    nc = bass.Bass(target_bir_lowering=False)
    build_attention_core(nc, SEQ, use_f32r=use_f32r)
    split_multi_waits(nc)
    return nc


def shard_inputs(x, wq, wk, wv, wo):
    """Full inputs -> per-core in_maps. Core c = (b = c//4, g = c%4)."""
    x = np.asarray(x, np.float32)
    wq = np.asarray(wq, np.float32)
    wk = np.asarray(wk, np.float32)
    wv = np.asarray(wv, np.float32)
    wo = np.asarray(wo, np.float32)
    ident = np.eye(128, dtype=np.float32)
    ones = np.ones((128, 64), np.float32)
    import ml_dtypes
    bf16 = ml_dtypes.bfloat16
    xTs = [np.ascontiguousarray(x[b].T).astype(bf16) for b in range(BSZ)]
    in_maps = []
    for c in range(N_CORES):
        b, g = c // 4, c % 4
        # fold the 1/sqrt(head_dim) score scaling into wq
        wq_g = (np.ascontiguousarray(wq[:, g * FH:(g + 1) * FH]) * (1.0 / np.sqrt(HD))).astype(bf16)
        wkv_g = np.ascontiguousarray(
            np.concatenate(
                [wk[:, g * HD:(g + 1) * HD], wv[:, g * HD:(g + 1) * HD]], axis=1
            )
        ).astype(bf16)
        wo_g = np.ascontiguousarray(wo[g * FH:(g + 1) * FH, :]).astype(bf16)
        in_maps.append(
            {"xT": xTs[b], "wq": wq_g, "wkv": wkv_g, "wo": wo_g,
             "ident": ident, "ones": ones}
        )
    return in_maps


def unshard_output(results):
    """Sum the 4 row-parallel partial outputs per batch."""
    out = np.zeros((BSZ, SEQ, DIM), np.float32)
    for c in range(N_CORES):
        out[c // 4] += np.asarray(results[c]["out"], np.float32)
    return out


_cache = {}


def kernel(x, wq, wk, wv, wo):
    if "nc" not in _cache:
        _cache["nc"] = build()
    nc = _cache["nc"]
    in_maps = shard_inputs(x, wq, wk, wv, wo)
    try:
        res = bass_utils.run_bass_kernel_spmd(
            nc, in_maps, core_ids=list(range(N_CORES))
        )
    except ModuleNotFoundError:
        # BASS_TRACE under an axon client without the NTFF hook module;
        # rerun untraced.
        import os

        os.environ["BASS_NEVER_TRACE"] = "1"
        res = bass_utils.run_bass_kernel_spmd(
            nc, in_maps, core_ids=list(range(N_CORES))
        )
    return unshard_output(res.results)

